# revision 3
# baseline (speedup 1.0000x reference)
import sys
if '/opt/trn_rl_repo' not in sys.path:
    sys.path.insert(0, '/opt/trn_rl_repo')
import numpy as np
import ml_dtypes

import concourse.bass as bass
import concourse.bacc as bacc
import concourse.tile as tile
from concourse import mybir
from concourse import bass_utils

f32 = mybir.dt.float32
f32r = mybir.dt.float32r
bf16 = mybir.dt.bfloat16
f8 = mybir.dt.float8e3          # e3m4: range +-15.5, 4 mantissa bits
f8np = ml_dtypes.float8_e3m4
FX = mybir.ActivationFunctionType
ALU = mybir.AluOpType
AX = mybir.AxisListType

B, D, H, DH = 256, 256, 8, 32
NCORES = 8
BC = B // NCORES          # 32 batches per core
LC = 1024                 # self-attn KV cache length
NA = 2048                 # cross-attn key count
KT_S = LC // 128          # 8 key tiles (self)
KT_A = NA // 128          # 16 key tiles (cross)
SCALE = 1.0 / float(np.sqrt(DH))
EPS = 1e-5
VROW = 272                # padded V row stride (16B aligned); col 256 = ones
VUSE = 260                # V cols consumed by the matmul (256 data + denom at 256, zero pad)
KVA = 2 * NA + KT_A * VROW   # 8448 combined bytes/partition, cross
KVS = 2 * LC + KT_S * VROW   # 4224 combined bytes/partition, self

WNAMES = ['wq_s', 'wk_s', 'wv_s', 'w0_s', 'wq_a', 'w0_a', 'w1', 'w2']
BNAMES = ['bq_s', 'bk_s', 'bv_s', 'b0_s', 'bq_a', 'b0_a', 'b1', 'b2']
LNAMES = ['ln1_g', 'ln1_b', 'ln2_g', 'ln2_b', 'ln3_g', 'ln3_b']


def _declare_dram(nc):
    dr = {}
    dr['h_t'] = nc.dram_tensor('h_t', [BC, 1, D], f32, kind='ExternalInput')
    # Host-packed fp8 K^T + V (+ones) per batch; one DMA per (batch, attn).
    # K^T region: [chunk c][block m][col p] = K[b, 2*(128*(m//2)+p)+(m%2), 128c+dd]
    # V region:   [block m][VROW] rows with col 256 = 1.0 (denominator)
    dr['KV_att'] = nc.dram_tensor('KV_att', [BC, 128, KVA], f8, kind='ExternalInput')
    dr['KV_cache'] = nc.dram_tensor('KV_cache', [BC, 128, KVS], f8, kind='ExternalInput')
    # keep-mask (1.0 = keep, 0.0 = masked), permuted like the V rows
    dr['notmT'] = nc.dram_tensor('notmT', [128, KT_A, BC], f32, kind='ExternalInput')
    dr['ident'] = nc.dram_tensor('ident', [128, 128], f32, kind='ExternalInput')
    for n in WNAMES:
        dr[n] = nc.dram_tensor(n, [D, D], f32r, kind='ExternalInput')
    for n in BNAMES + LNAMES:
        dr[n] = nc.dram_tensor(n, [D], f32, kind='ExternalInput')
    dr['out'] = nc.dram_tensor('out', [BC, D], f32, kind='ExternalOutput')
    return dr


def _build():
    nc = bacc.Bacc()
    dr = _declare_dram(nc)
    out = dr.pop('out')
    with tile.TileContext(nc) as tc:
        _emit(nc, tc, dr, out)
    nc.compile()
    return nc


def _emit(nc, tc, dr, out_dram):
    import contextlib
    ctx = contextlib.ExitStack()
    with ctx:
        const = ctx.enter_context(tc.tile_pool(name='const', bufs=1))
        kv_p = ctx.enter_context(tc.tile_pool(name='kv', bufs=5))
        ex_p = ctx.enter_context(tc.tile_pool(name='ex', bufs=2))
        sm_p = ctx.enter_context(tc.tile_pool(name='sm', bufs=4))
        tr_ps = ctx.enter_context(tc.tile_pool(name='trps', bufs=2, space='PSUM'))
        sc_ps = ctx.enter_context(tc.tile_pool(name='scps', bufs=2, space='PSUM'))
        at_ps = ctx.enter_context(tc.tile_pool(name='atps', bufs=2, space='PSUM'))
        ln_ps = ctx.enter_context(tc.tile_pool(name='lnps', bufs=1, space='PSUM'))
        gb_ps = ctx.enter_context(tc.tile_pool(name='gbps', bufs=1, space='PSUM'))

        garb = gb_ps.tile([1, 1], f32, tag='garb')
        last_act = [None]

        def pe_absorb(*aps):
            # PE matmul/transpose (fp32/fp32r self-loading weights) can carry only ONE
            # sem wait in its LW slot. Before a matmul whose deps span several procs,
            # emit 1x1 self-matmuls so the PE observes those sems here instead.
            for a in aps:
                if a is None:
                    continue
                e = a[tuple(slice(0, 1) for _ in range(len(a.shape)))]
                if e.dtype == f32r:
                    e = e.bitcast(f32)
                nc.tensor.matmul(garb[:, :], e, e, start=True, stop=True,
                                 skip_group_check=True)

        # ---------- persistent loads ----------
        ident = const.tile([128, 128], f32, tag='ident')
        nc.gpsimd.dma_start(out=ident, in_=dr['ident'][:, :])
        pe_absorb(ident)
        ht = const.tile([BC, D], f32, tag='ht')
        nc.gpsimd.dma_start(out=ht, in_=dr['h_t'][:, 0, :])
        pe_absorb(ht)
        epst = const.tile([BC, 1], f32, tag='epst')
        nc.vector.memset(epst, EPS)

        wsb = {}
        for n in WNAMES:
            wsb[n] = const.tile([128, 2, D], f32r, tag='w_' + n, name='w_' + n)
            nc.gpsimd.dma_start(out=wsb[n], in_=dr[n][:, :].rearrange('(t p) j -> p t j', p=128))
        vsb = {}
        for n in BNAMES + LNAMES:
            vsb[n] = const.tile([BC, D], f32, tag='v_' + n, name='v_' + n)
            nc.gpsimd.dma_start(out=vsb[n], in_=dr[n][:].unsqueeze(0).to_broadcast([BC, D]))

        notmT = const.tile([128, KT_A, BC], f32, tag='notmT')
        nc.gpsimd.dma_start(out=notmT, in_=dr['notmT'][:, :, :])
        # ping-pong exp-weight tiles, zero-padded to 32 cols so the col-tiled
        # V matmuls initialize full 32-partition PSUM groups
        wexPP = []
        for i in range(2):
            w = const.tile([128, KT_A, 32], f8, tag='wex%d' % i, name='wex%d' % i)
            nc.vector.memset(w, 0.0)
            wexPP.append(w)

        # ---------- helpers ----------
        def transpose_128(dst, src, cols):
            # src [rows<=128, cols<=128] SBUF f32 -> dst [cols, rows] via PE transpose
            rows = src.shape[0]
            ps = tr_ps.tile([128, 128], f32, tag='trps')
            nc.tensor.transpose(ps[0:cols, 0:rows], src, ident[0:rows, 0:rows])
            nc.vector.tensor_copy(out=dst, in_=ps[0:cols, 0:rows])

        def make_T(src_f32, tagname):
            # src [BC, D] -> [128, 2, BC] f32r transposed halves
            dstT = const.tile([128, 2, BC], f32r, tag=tagname, name=tagname)
            for t in range(2):
                transpose_128(dstT[:, t, :], src_f32[:, 128 * t:128 * (t + 1)], 128)
            return dstT

        def linear_psum(srcT_list, wname):
            # sum_t sum_s srcT.T @ W  -> psum [BC, D]
            ps = ln_ps.tile([BC, D], f32, tag='lnps')
            pe_absorb(wsb[wname])
            n_mm = 2 * len(srcT_list)
            i = 0
            for srcT in srcT_list:
                for t in range(2):
                    nc.tensor.matmul(ps[:, :], srcT[:, t, :], wsb[wname][:, t, :],
                                     start=(i == 0), stop=(i == n_mm - 1))
                    i += 1
            return ps

        def layernorm(dst, src, gname, bname, tagp):
            stats = const.tile([BC, 6], f32, tag=tagp + '_st', name=tagp + '_st')
            nc.vector.bn_stats(out=stats, in_=src)
            mv = const.tile([BC, 2], f32, tag=tagp + '_mv', name=tagp + '_mv')
            nc.vector.bn_aggr(out=mv, in_=stats)
            sd = const.tile([BC, 1], f32, tag=tagp + '_sd', name=tagp + '_sd')
            nc.scalar.activation(out=sd, in_=mv[:, 1:2], func=FX.Sqrt,
                                 bias=epst[:, :], scale=1.0)
            rstd = const.tile([BC, 1], f32, tag=tagp + '_rs', name=tagp + '_rs')
            nc.vector.reciprocal(out=rstd, in_=sd)
            nc.vector.tensor_scalar(out=dst, in0=src, scalar1=mv[:, 0:1], scalar2=rstd,
                                    op0=ALU.subtract, op1=ALU.mult)
            nc.vector.tensor_mul(dst, dst, vsb[gname])
            nc.vector.tensor_add(dst, dst, vsb[bname])

        def build_qblk(qsrc_f32, tagp):
            # -> [128, 2, BC, H] fp8, block-diagonal per head (zeros elsewhere)
            qT = make_T(qsrc_f32, tagp + '_qT')
            qb = const.tile([128, 2, BC, H], f8, tag=tagp + '_qb', name=tagp + '_qb')
            nc.vector.memset(qb, 0.0)
            for t in range(2):
                for hh in range(4):
                    h = 4 * t + hh
                    nc.vector.tensor_copy(out=qb[32 * hh:32 * (hh + 1), t, :, h],
                                          in_=qT[32 * hh:32 * (hh + 1), t, :])
            return qb

        # ---------- qkv for self-attn ----------
        htT = make_T(ht, 'htT')
        qkv = {}
        for nm, wn, bn in (('q', 'wq_s', 'bq_s'), ('k', 'wk_s', 'bk_s'), ('v', 'wv_s', 'bv_s')):
            ps = linear_psum([htT], wn)
            qkv[nm] = const.tile([BC, D], f32, tag='qkv_' + nm, name='qkv_' + nm)
            nc.vector.tensor_add(qkv[nm], ps, vsb[bn])

        qblk_s = build_qblk(qkv['q'], 'self')

        # new-key (appended k/v) terms, all-batch
        qk = const.tile([BC, D], f32, tag='qk')
        nc.vector.tensor_mul(qk, qkv['q'], qkv['k'])
        s_new = const.tile([BC, H], f32, tag='s_new')
        nc.vector.reduce_sum(out=s_new, in_=qk.rearrange('p (g s) -> p g s', g=H), axis=AX.X)
        w_new = const.tile([BC, H], f32, tag='w_new')
        nc.scalar.activation(out=w_new, in_=s_new, func=FX.Exp, scale=SCALE)
        w_newT = const.tile([H, BC], f32, tag='w_newT')
        pe_absorb(w_new)
        transpose_128(w_newT, w_new, H)

        invmix = const.tile([H, BC], f32, tag='invmix')

        # ---------- attention inner loop ----------
        def attention(qblk, n_tiles, KV_dram, attT_dst, masked, inv_store):
            ksz = n_tiles * 128           # K^T bytes per chunk per partition
            voff = 2 * ksz                # V region offset
            tot = voff + n_tiles * VROW
            for bp in range(BC // 2):
                # one 2-batch DMA, alternating between the two HWDGE rings
                kvp = kv_p.tile([128, 2, tot], f8, tag='kv%d' % tot)
                eng = nc.sync if bp % 2 == 0 else nc.scalar
                eng.dma_start(out=kvp[:, :, :],
                              in_=KV_dram[2 * bp:2 * bp + 2].rearrange('b p x -> p b x'))
                pe_absorb(kvp)
                for j in range(2):
                    b = 2 * bp + j
                    kv = kvp[:, j, :]
                    _attn_batch(qblk, n_tiles, kv, ksz, voff, attT_dst, masked,
                                inv_store, b)

        def _attn_batch(qblk, n_tiles, kv, ksz, voff, attT_dst, masked, inv_store, b):
            if True:
                # scores: s^T [key, head] per 128-key block, both d-chunks accumulated
                sps = sc_ps.tile([128, KT_A, H], f32, tag='scps')
                if last_act[0] is not None:
                    pe_absorb(last_act[0])
                for t in range(n_tiles):
                    nc.tensor.matmul(sps[:, t, :], kv[:, 128 * t:128 * (t + 1)],
                                     qblk[:, 0, b, :], start=True, stop=False)
                    nc.tensor.matmul(sps[:, t, :], kv[:, ksz + 128 * t:ksz + 128 * (t + 1)],
                                     qblk[:, 1, b, :], start=False, stop=True)
                # exp (+ mask via multiply) -> fp8 weights
                wex = wexPP[b % 2]
                if masked:
                    ex1 = ex_p.tile([128, KT_A, H], f32, tag='ex')
                    nc.scalar.activation(out=ex1[:, 0:n_tiles, :], in_=sps[:, 0:n_tiles, :],
                                         func=FX.Exp, scale=SCALE)
                    nc.vector.tensor_tensor(
                        out=wex[:, 0:n_tiles, 0:H], in0=ex1[:, 0:n_tiles, :],
                        in1=notmT[:, 0:n_tiles, b:b + 1].broadcast_to([128, n_tiles, H]),
                        op=ALU.mult)
                else:
                    nc.scalar.activation(out=wex[:, 0:n_tiles, 0:H], in_=sps[:, 0:n_tiles, :],
                                         func=FX.Exp, scale=SCALE)
                last_act[0] = wex[:, 0:1, 0:H]
                # weighted V (+ denominator from the baked-in ones column)
                atp = at_ps.tile([H, VUSE], f32, tag='atps')
                pe_absorb(wex[:, 0:1, 0:1])
                for t in range(n_tiles):
                    nc.tensor.matmul(atp[:, :], wex[:, t, 0:H],
                                     kv[:, voff + VROW * t:voff + VROW * t + VUSE],
                                     start=(t == 0), stop=(t == n_tiles - 1),
                                     skip_group_check=True)
                # denominator -> inverse
                dn = sm_p.tile([H, 1], f32, tag='dn')
                if inv_store is not None:
                    nc.vector.tensor_add(dn, atp[:, D:D + 1], w_newT[:, b:b + 1])
                else:
                    nc.vector.tensor_copy(out=dn, in_=atp[:, D:D + 1])
                iv = sm_p.tile([H, 1], f32, tag='iv')
                nc.vector.reciprocal(out=iv, in_=dn)
                if inv_store is not None:
                    nc.vector.tensor_copy(out=inv_store[:, b:b + 1], in_=iv)
                # scaled mixed attention, then un-mix via transpose + 32-aligned copies
                attm = sm_p.tile([H, D], f32, tag='attm')
                nc.vector.tensor_scalar_mul(out=attm, in0=atp[:, 0:D], scalar1=iv)
                for t in range(2):
                    pa = tr_ps.tile([128, 128], f32, tag='trps')
                    nc.tensor.transpose(pa[0:128, 0:H], attm[:, 128 * t:128 * (t + 1)],
                                        ident[0:H, 0:H])
                    for k in range(4):
                        h = 4 * t + k
                        nc.vector.tensor_copy(out=attT_dst[32 * k:32 * (k + 1), t, b:b + 1],
                                              in_=pa[32 * k:32 * (k + 1), h:h + 1])

        # ---------- self attention ----------
        attT_s = const.tile([128, 2, BC], f32r, tag='attT_s')
        attention(qblk_s, KT_S, dr['KV_cache'], attT_s, False, invmix)

        # new-key numerator: nv = v * w_new * inv  (batch layout), then transpose
        invb = const.tile([BC, H], f32, tag='invb')
        transpose_128(invb, invmix, BC)
        nv = const.tile([BC, D], f32, tag='nv')
        nc.vector.tensor_tensor(out=nv.rearrange('p (g s) -> p g s', g=H),
                                in0=qkv['v'].rearrange('p (g s) -> p g s', g=H),
                                in1=w_new.unsqueeze(2).broadcast_to([BC, H, DH]),
                                op=ALU.mult)
        nc.vector.tensor_tensor(out=nv.rearrange('p (g s) -> p g s', g=H),
                                in0=nv.rearrange('p (g s) -> p g s', g=H),
                                in1=invb.unsqueeze(2).broadcast_to([BC, H, DH]),
                                op=ALU.mult)
        nvT = make_T(nv, 'nvT')

        # h1 = LN1(ht + att_self @ w0_s + b0_s)
        ps = linear_psum([attT_s, nvT], 'w0_s')
        h1p = const.tile([BC, D], f32, tag='h1p')
        nc.vector.tensor_add(h1p, ps, vsb['b0_s'])
        nc.vector.tensor_add(h1p, h1p, ht)
        h1 = const.tile([BC, D], f32, tag='h1')
        layernorm(h1, h1p, 'ln1_g', 'ln1_b', 'ln1')

        # ---------- cross attention ----------
        h1T = make_T(h1, 'h1T')
        psq = linear_psum([h1T], 'wq_a')
        qa = const.tile([BC, D], f32, tag='qa')
        nc.vector.tensor_add(qa, psq, vsb['bq_a'])
        qblk_a = build_qblk(qa, 'cross')

        attT_a = const.tile([128, 2, BC], f32r, tag='attT_a')
        attention(qblk_a, KT_A, dr['KV_att'], attT_a, True, None)

        # h2 = LN2(h1 + att_cross @ w0_a + b0_a)
        ps2 = linear_psum([attT_a], 'w0_a')
        h2p = const.tile([BC, D], f32, tag='h2p')
        nc.vector.tensor_add(h2p, ps2, vsb['b0_a'])
        nc.vector.tensor_add(h2p, h2p, h1)
        h2 = const.tile([BC, D], f32, tag='h2')
        layernorm(h2, h2p, 'ln2_g', 'ln2_b', 'ln2')

        # ---------- MLP ----------
        h2T = make_T(h2, 'h2T')
        psm = linear_psum([h2T], 'w1')
        m1 = const.tile([BC, D], f32, tag='m1')
        nc.vector.tensor_add(m1, psm, vsb['b1'])
        m1r = const.tile([BC, D], f32, tag='m1r')
        nc.scalar.activation(out=m1r, in_=m1, func=FX.Relu, scale=1.0)
        pe_absorb(m1r)
        m1T = make_T(m1r, 'm1T')
        psm2 = linear_psum([m1T], 'w2')
        h3p = const.tile([BC, D], f32, tag='h3p')
        nc.vector.tensor_add(h3p, psm2, vsb['b2'])
        nc.vector.tensor_add(h3p, h3p, h2)
        outt = const.tile([BC, D], f32, tag='outt')
        layernorm(outt, h3p, 'ln3_g', 'ln3_b', 'ln3')
        nc.sync.dma_start(out=out_dram[:, :], in_=outt)


_CACHE = {}


def _get_nc():
    if 'nc' not in _CACHE:
        _CACHE['nc'] = _build()
    return _CACHE['nc']


def _pack_kv(K, V):
    # K, V: [BC, N, D] float arrays -> [BC, 128, 2*N + (N//128)*VROW] fp8
    # key permutation: block m (of 128 keys), col/row p <-> key 2*(128*(m//2)+p)+(m%2)
    n = K.shape[1]
    nt = n // 128
    k8 = K.astype(f8np).reshape(BC, nt // 2, 128, 2, D)          # [b, t, p, j, d]
    kp = np.ascontiguousarray(k8.transpose(0, 4, 1, 3, 2))       # [b, d, t, j, p]
    kp = kp.reshape(BC, 2, 128, nt * 128).transpose(0, 2, 1, 3)  # [b, dd, c, cols]
    kp = np.ascontiguousarray(kp).reshape(BC, 128, 2 * n)
    v8 = V.astype(f8np).reshape(BC, nt // 2, 128, 2, D)          # [b, t, p, j, d]
    vp = np.zeros((BC, 128, nt, VROW), dtype=f8np)
    vp[:, :, :, 0:D] = v8.transpose(0, 2, 1, 3, 4).reshape(BC, 128, nt, D)
    vp[:, :, :, D] = np.asarray(1.0, dtype=f8np)
    return np.concatenate([kp, vp.reshape(BC, 128, nt * VROW)], axis=2)


def _make_in_maps(inputs):
    np_in = {k: np.asarray(v) for k, v in inputs.items()}
    ident = np.eye(128, dtype=np.float32)
    in_maps = []
    for c in range(NCORES):
        sl = slice(c * BC, (c + 1) * BC)
        m = np_in['mask'][sl].astype(np.float32)          # [BC, NA], True = masked
        # keep-mask, permuted to match the packed key order: [p, m, b]
        notm = (1.0 - m).reshape(BC, KT_A // 2, 128, 2).transpose(2, 1, 3, 0)
        notm = np.ascontiguousarray(notm).reshape(128, KT_A, BC)
        im = {
            'h_t': np.ascontiguousarray(np_in['h_t'][sl]),
            'KV_att': _pack_kv(np_in['K_att'][sl], np_in['V_att'][sl]),
            'KV_cache': _pack_kv(np_in['K_cache'][sl], np_in['V_cache'][sl]),
            'notmT': notm,
            'ident': ident,
        }
        for n in WNAMES + BNAMES + LNAMES:
            im[n] = np.ascontiguousarray(np_in[n])
        in_maps.append(im)
    return in_maps


def run_on_device(inputs):
    nc = _get_nc()
    in_maps = _make_in_maps(inputs)
    res = bass_utils.run_bass_kernel_spmd(nc, in_maps, core_ids=list(range(NCORES)),
                                          trace=False)
    outs = [res.results[c]['out'] for c in range(NCORES)]
    return np.concatenate(outs, axis=0).astype(np.float32)


def kernel(**inputs):
    return run_on_device(inputs)
